# revision 8
# baseline (speedup 1.0000x reference)
"""Bass/Tile kernel for chunkwise retention (nn_ChunkwiseRetention).

Shifted-window scheme (v2), per core = one batch element, seq 4000, B=5:

Windows of 125 positions shifted by -5: window s covers output positions
[s*125-5, s*125+120), and the V/K contraction range is the SAME shifted
span, so the seam (intra of the chunk straddling the superchunk boundary)
folds into the single combined masked matmul — no separate seam matmul.
The carry boundary moves one chunk earlier: carry_s = Q[s*125:+125] @
U_shift(s-1) with U_shift accumulating K^T V over shifted windows.

Host pre-scales xqT columns by g6^j and xkT by g6^-j (j = chunk index),
folding all cross-chunk decay into the projections (cross mask is 0/1).
All inputs and SBUF operands are bf16 (PE: 1 cycle/row at any moving
width, halved DMA); PSUM accumulation stays f32.

Per iteration s: V proj (s+1, shifted window), window matmuls for s
(comb + carry into one PSUM group), P~^T (s+1) at N=130 (q cols shifted
-5..+125), state update, group-ahead Q^T/K^T projections (N=505, fused
single PSUM->SBUF copy), K pos-major via PE transposes into a bf16
bitcast region of the same PSUM tile as P~^T (shared bank, bufs=2 so
the mask chain has a full iteration of slack), one DVE mask mul via a
2-block strided AP + SBUF-only add on the Pool engine, paired output
DMAs. Tail (chunk 799, intra-only) is issued early at s==27.

PSUM banks (8): qkt 2 + v 1 + (pt|ktr) 2 + wt 2 + u 1.
"""
import numpy as np
import ml_dtypes

import concourse.bass as bass
import concourse.mybir as mybir
import concourse.tile as tile

GAMMA = 0.9865
B = 5
SEQ = 4000
FEAT = 256
DIM = 256
GP = 125              # window size (25 chunks)
NSC = SEQ // GP       # 32
NG = 8                # groups of 4 windows
GW = 505              # group buffer width (500 + 5 shift overlap)
F32 = mybir.dt.float32
F32R = mybir.dt.float32r
BF16 = mybir.dt.bfloat16
g6 = float(np.float64(GAMMA) ** 6)
COPY = mybir.ActivationFunctionType.Copy

# const blob column layout (f32)
C_WIT = 0             # [0:125)   intra mask, shifted coords
C_WCT = 125           # [125:250) 0/1 cross mask, shifted coords
C_Z = 250             # [250:762) zeros (row 0: zero matmul operands)
C_END = 762


def make_const_blob():
    j = np.arange(GP)
    jj, rr = j[:, None], j[None, :]
    witn = np.where((jj // B == rr // B) & (rr % B >= jj % B),
                    np.float64(GAMMA) ** (jj % B - rr % B), 0.0)
    wctn = (jj // B <= rr // B).astype(np.float64)
    blob = np.zeros((128, C_END), np.float32)
    blob[0:GP, C_WIT:C_WIT + GP] = witn.astype(np.float32)
    blob[0:GP, C_WCT:C_WCT + GP] = wctn.astype(np.float32)
    return blob


def build_kernel(nc: bass.Bass):
    xqT = nc.dram_tensor("xqT", [FEAT, SEQ], BF16, kind="ExternalInput").ap()
    xkT = nc.dram_tensor("xkT", [FEAT, SEQ], BF16, kind="ExternalInput").ap()
    xvT = nc.dram_tensor("xvT", [FEAT, SEQ], BF16, kind="ExternalInput").ap()
    wqkv = nc.dram_tensor("wqkv", [FEAT, 3 * DIM], BF16, kind="ExternalInput").ap()
    out = nc.dram_tensor("out", [SEQ, DIM], F32, kind="ExternalOutput").ap()

    blob_np = make_const_blob()
    ident_np = np.eye(128, dtype=ml_dtypes.bfloat16)
    mm = nc.tensor.matmul

    with tile.TileContext(nc) as tc:
        with (
            tc.tile_pool(name="consts", bufs=1) as cpool,
            tc.tile_pool(name="xin", bufs=3) as xpool,
            tc.tile_pool(name="qkt", bufs=2) as qpool,
            tc.tile_pool(name="work", bufs=2) as spool,
            tc.tile_pool(name="psT", bufs=1, space="PSUM") as psT,
            tc.tile_pool(name="psV", bufs=1, space="PSUM") as psV,
            tc.tile_pool(name="psX", bufs=2, space="PSUM") as psX,
            tc.tile_pool(name="psW", bufs=2, space="PSUM") as psW,
            tc.tile_pool(name="psU", bufs=1, space="PSUM") as psU,
        ):
            # weights DMA first (first projection gates on it), then group-0
            # x loads, then the const blob / identity, then group 1.
            w_sb = cpool.tile_from(wqkv.rearrange("(h p) d -> p h d", p=128))
            blob_sb = cpool.tile([128, C_END], F32, name="blob_sb")
            ident_sb = cpool.tile([128, 128], BF16, name="ident_sb")
            wit_sb = blob_sb[0:GP, C_WIT:C_WIT + GP]
            ww_sb = blob_sb[0:GP, 0:2 * GP].rearrange("p (b c) -> p b c", b=2)

            u_ps = psU.tile([128, 512], F32, name="u_state")

            xg = {}
            qts = {}
            kts = {}

            def load_group_x(g):
                tiles = []
                for nm, src in (("xq", xqT), ("xk", xkT), ("xv", xvT)):
                    t = xpool.tile([128, 2, GW], BF16, name=f"{nm}_{g}", tag=nm)
                    if g == 0:
                        nc.vector.memset(t[:, :, 0:5], 0.0)
                        nc.sync.dma_start(
                            out=t[:, :, 5:GW],
                            in_=src[:, 0:500].rearrange("(h p) a -> p h a", p=128))
                    else:
                        nc.sync.dma_start(
                            out=t,
                            in_=src[:, g * 500 - 5:g * 500 + 500]
                                .rearrange("(h p) a -> p h a", p=128))
                    tiles.append(t)
                xg[g] = tiles

            def proj_qkt(g, which):
                x = xg[g][0 if which == "qt" else 1]
                dlos = (0, 128) if which == "qt" else (256, 384)
                ps = psT.tile([128, 1024], F32, name=f"ps_{which}_{g}", tag="qkt")
                for off, dlo in ((0, dlos[0]), (512, dlos[1])):
                    for h in (0, 1):
                        mm(ps[:, off:off + GW], w_sb[:, h, dlo:dlo + 128],
                           x[:, h, :], start=(h == 0), stop=(h == 1))
                t = qpool.tile([128, 2, GW], BF16, name=f"{which}_{g}", tag=which)
                ps_v = ps.rearrange("p (b c) -> p b c", b=2)[:, :, 0:GW]
                nc.scalar.activation(t, ps_v, COPY)
                (qts if which == "qt" else kts)[g] = t

            def vproj(s):
                g, ls = divmod(s, 4)
                xv = xg[g][2]
                vs = psV.tile([GP, 256], F32, name=f"v_{s}", tag="v")
                for h in (0, 1):
                    mm(vs, xv[:, h, ls * GP:ls * GP + GP], w_sb[:, h, 512:768],
                       start=(h == 0), stop=(h == 1))
                v_sb = spool.tile([GP, 256], BF16, name=f"vsb_{s}", tag="vsb")
                nc.vector.tensor_copy(v_sb, vs)
                return v_sb

            def ptmm(s):
                # P~^T in cols 0:130 (f32) of a [125, 512] tile whose bytes
                # 1536:2048 also hold the K-transpose output (bf16 bitcast) —
                # one bank, two bufs, so the mask chain isn't serialized.
                g, ls = divmod(s, 4)
                qt, kt = qts[g], kts[g]
                px = psX.tile([GP, 512], F32, name=f"px_{s}", tag="px")
                for h in (0, 1):
                    mm(px[:, 0:130], kt[:, h, ls * GP:ls * GP + GP],
                       qt[:, h, ls * GP:ls * GP + 130],
                       start=(h == 0), stop=(h == 1))
                return px

            def ktrans(s, px):
                g, ls = divmod(s, 4)
                kt = kts[g]
                kv = px[:, 384:512].bitcast(BF16)   # [125, 256] bf16 region
                mm(kv[:, 0:128], kt[:, 0, ls * GP:ls * GP + GP], ident_sb,
                   is_transpose=True, skip_group_check=True)
                mm(kv[:, 128:256], kt[:, 1, ls * GP:ls * GP + GP], ident_sb,
                   is_transpose=True, skip_group_check=True)
                k_sb = spool.tile([GP, 256], BF16, name=f"ksb_{s}", tag="ksb",
                                  bufs=3)
                if s % 2 == 0:
                    nc.vector.tensor_copy(k_sb, kv)
                else:
                    nc.scalar.activation(k_sb, kv, COPY)
                return k_sb

            def masks(s, px):
                # one DVE mul: in0 = pt at col offsets {0, 5} (2-block AP),
                # in1 = [wit | wct] blob block, out = both products; the
                # SBUF-only add runs on the otherwise-idle Pool engine.
                c2 = spool.tile([GP, 2, GP], BF16, name=f"c2_{s}", tag="c2")
                pt_blocks = bass.AP(px.tensor, px.offset,
                                    [[512, GP], [5, 2], [1, GP]])
                nc.vector.tensor_mul(c2, pt_blocks, ww_sb)
                comb = spool.tile([GP, GP], BF16, name=f"comb_{s}", tag="comb",
                                  bufs=3)
                nc.gpsimd.tensor_add(comb, c2[:, 0, :], c2[:, 1, :])
                return comb

            def tail():
                # chunk 799 (positions 3995:4000), intra-only
                xv7 = xg[7][2]
                v5ps = psV.tile([5, 256], F32, name="v5", tag="v")
                for h in (0, 1):
                    mm(v5ps, xv7[:, h, 500:GW], w_sb[:, h, 512:768],
                       start=(h == 0), stop=(h == 1))
                v5_sb = spool.tile([5, 256], BF16, name="v5sb", tag="v5sb")
                nc.vector.tensor_copy(v5_sb, v5ps)
                px5 = psX.tile([GP, 512], F32, name="px5", tag="px")
                for h in (0, 1):
                    mm(px5[0:5, 0:5], kts[7][:, h, 500:GW],
                       qts[7][:, h, 500:GW], start=(h == 0), stop=(h == 1))
                c5 = spool.tile([5, 5], BF16, name="c5", tag="c5")
                nc.vector.tensor_mul(c5, px5[0:5, 0:5], blob_sb[0:5, C_WIT:C_WIT + 5])
                wtf = psW.tile([5, 256], F32, name="wtf", tag="wt")
                mm(wtf, c5, v5_sb, start=True, stop=True)
                wallf = spool.tile([5, 256], F32, name="wallf", tag="wallf")
                nc.scalar.activation(wallf, wtf, COPY)
                nc.sync.dma_start(out=out[SEQ - 5:SEQ], in_=wallf)

            # --- prologue ---
            load_group_x(0)
            nc.sync.dma_start(out=blob_sb, in_=nc.inline_tensor(blob_np, "cblob").ap())
            nc.sync.dma_start(out=ident_sb,
                              in_=nc.inline_tensor(ident_np, "cident").ap())
            load_group_x(1)

            # preamble: absorb const/weight DMA waits; zero-matmul sets the
            # U bank's data + has_written bits so state matmuls accumulate.
            mm(u_ps[0:1, 0:1], w_sb[:, 0, 0:1], w_sb[:, 0, 0:1],
               start=True, stop=True, skip_group_check=True)
            scr = spool.tile([1, 1], F32, name="scr", tag="scr")
            nc.vector.tensor_copy(scr, blob_sb[0:1, 0:1])
            scr2 = spool.tile([1, 1], BF16, name="scr2", tag="scr2")
            nc.scalar.activation(scr2, ident_sb[0:1, 0:1], COPY)
            mm(u_ps, blob_sb[0:1, C_Z:C_Z + 128].bitcast(F32R),
               blob_sb[0:1, C_Z:C_Z + 512].bitcast(F32R),
               start=True, stop=True, skip_group_check=True)

            proj_qkt(0, "qt")
            proj_qkt(0, "kt")
            vs = {0: vproj(0)}
            ks = {}
            combs = {}
            for t in (0, 1):
                pxt = ptmm(t)
                ks[t] = ktrans(t, pxt)
                combs[t] = masks(t, pxt)
            ut_prev = None
            wall_pair = {}

            for s in range(NSC):
                g, ls = divmod(s, 4)
                # P~^T(s+2) + its DVE mask mul go FIRST (PE and DVE program
                # order) so the mask chain for comb(s+2) starts two
                # iterations early and never gates a window matmul.
                px = ptmm(s + 2) if s + 2 < NSC else None
                if px is not None:
                    combs[s + 2] = masks(s + 2, px)
                if s + 1 < NSC:
                    vs[s + 1] = vproj(s + 1)

                wt = psW.tile([GP, 256], F32, name=f"wt_{s}", tag="wt")
                mm(wt, combs[s], vs[s], start=True, stop=(s == 0))
                if s > 0:
                    qt = qts[g]
                    mm(wt, qt[:, 0, ls * GP + 5:ls * GP + 130], ut_prev[:, 0:256],
                       start=False, stop=False)
                    mm(wt, qt[:, 1, ls * GP + 5:ls * GP + 130],
                       ut_prev[:, 256:512], start=False, stop=True)

                mm(u_ps[:, 0:256], ks[s][:, 0:128], vs[s],
                   start=False, stop=True, skip_group_check=True)
                mm(u_ps[:, 256:512], ks[s][:, 128:256], vs[s],
                   start=False, stop=True, skip_group_check=True)
                if s + 1 < NSC:
                    ut_prev = spool.tile([128, 512], BF16, name=f"ut_{s}", tag="ut")
                    nc.scalar.activation(ut_prev, u_ps, COPY)

                if ls == 0 and g + 1 < NG:
                    proj_qkt(g + 1, "qt")
                if ls == 1 and g + 1 < NG:
                    proj_qkt(g + 1, "kt")
                    if g + 2 < NG:
                        load_group_x(g + 2)

                if px is not None:
                    ks[s + 2] = ktrans(s + 2, px)
                if s == 26:
                    tail()

                if s < 2:
                    wall = spool.tile([GP, 256], F32, name=f"wall_{s}",
                                      tag="wall0", bufs=2)
                    nc.vector.tensor_copy(wall, wt)
                    if s == 0:
                        nc.sync.dma_start(out=out[0:GP - 5], in_=wall[5:GP])
                    else:
                        nc.sync.dma_start(out=out[s * GP - 5:s * GP + 120],
                                          in_=wall)
                else:
                    if s % 2 == 0:
                        wall2 = spool.tile([GP, 2, 256], F32, name=f"wall_{s}",
                                           tag="wall", bufs=2)
                        wall_pair[0] = wall2
                        nc.vector.tensor_copy(wall2[:, 0, :], wt)
                    else:
                        wall2 = wall_pair[0]
                        nc.vector.tensor_copy(wall2[:, 1, :], wt)
                        nc.sync.dma_start(
                            out=out[(s - 1) * GP - 5:(s + 1) * GP - 5]
                                .rearrange("(b p) d -> p b d", b=2),
                            in_=wall2)
                vs.pop(s, None)
                ks.pop(s, None)
                combs.pop(s, None)

    return nc


def _col_scales():
    j = np.arange(SEQ) // B          # global chunk index
    sq = (np.float64(g6) ** j).astype(np.float32)
    sk = (np.float64(g6) ** (-j)).astype(np.float32)
    return sq, sk


def prep_core_inputs(xq2d, xk2d, xv2d, wqkv):
    sq, sk = _col_scales()
    return {
        "xqT": (xq2d.T * sq[None, :]).astype(ml_dtypes.bfloat16),
        "xkT": (xk2d.T * sk[None, :]).astype(ml_dtypes.bfloat16),
        "xvT": np.ascontiguousarray(xv2d.T).astype(ml_dtypes.bfloat16),
        "wqkv": wqkv.astype(ml_dtypes.bfloat16),
    }


def make_in_maps(inputs):
    """inputs: dict from setup_inputs (full batch). Returns per-core in_maps."""
    xq, xk, xv = inputs["xq"], inputs["xk"], inputs["xv"]
    wqkv = np.ascontiguousarray(np.concatenate(
        [np.asarray(inputs["Wq"], dtype=np.float32),
         np.asarray(inputs["Wk"], dtype=np.float32),
         np.asarray(inputs["Wv"], dtype=np.float32)], axis=1))
    in_maps = []
    for b in range(8):
        in_maps.append(prep_core_inputs(
            np.asarray(xq[b], dtype=np.float32),
            np.asarray(xk[b], dtype=np.float32),
            np.asarray(xv[b], dtype=np.float32), wqkv))
    return in_maps


_NC_CACHE = {}


def _get_nc():
    if "nc" not in _NC_CACHE:
        from concourse import bacc
        nc = bacc.Bacc("TRN2", target_bir_lowering=False, debug=False)
        build_kernel(nc)
        nc.compile()
        _NC_CACHE["nc"] = nc
    return _NC_CACHE["nc"]


def run(inputs, trace=False, **kwargs):
    """Run on 8 NeuronCores; returns (output [8,4000,256], BassKernelResults)."""
    from concourse.bass_utils import run_bass_kernel_spmd

    nc = _get_nc()
    in_maps = make_in_maps(inputs)
    res = run_bass_kernel_spmd(nc, in_maps, core_ids=list(range(8)),
                               trace=trace, **kwargs)
    out = np.stack([r["out"] for r in res.results], axis=0)
    return out, res


def kernel(**inputs) -> np.ndarray:
    out, _ = run(inputs)
    return out


# revision 10
# speedup vs baseline: 1.0421x; 1.0421x over previous
"""Bass/Tile kernel for chunkwise retention (nn_ChunkwiseRetention).

Shifted-window scheme (v2), per core = one batch element, seq 4000, B=5:

Windows of 125 positions shifted by -5: window s covers output positions
[s*125-5, s*125+120), and the V/K contraction range is the SAME shifted
span, so the seam (intra of the chunk straddling the superchunk boundary)
folds into the single combined masked matmul — no separate seam matmul.
The carry boundary moves one chunk earlier: carry_s = Q[s*125:+125] @
U_shift(s-1) with U_shift accumulating K^T V over shifted windows.

Host pre-scales xqT columns by g6^j and xkT by g6^-j (j = chunk index),
folding all cross-chunk decay into the projections (cross mask is 0/1).
All inputs and SBUF operands are bf16 (PE: 1 cycle/row at any moving
width, halved DMA); PSUM accumulation stays f32.

Per iteration s: V proj (s+1, shifted window), window matmuls for s
(comb + carry into one PSUM group), P~^T (s+1) at N=130 (q cols shifted
-5..+125), state update, group-ahead Q^T/K^T projections (N=505, fused
single PSUM->SBUF copy), K pos-major via PE transposes into a bf16
bitcast region of the same PSUM tile as P~^T (shared bank, bufs=2 so
the mask chain has a full iteration of slack), one DVE mask mul via a
2-block strided AP + SBUF-only add on the Pool engine, paired output
DMAs. Tail (chunk 799, intra-only) is issued early at s==27.

PSUM banks (8): qkt 2 + v 1 + (pt|ktr) 2 + wt 2 + u 1.
"""
import numpy as np
import ml_dtypes

import concourse.bass as bass
import concourse.mybir as mybir
import concourse.tile as tile

GAMMA = 0.9865
B = 5
SEQ = 4000
FEAT = 256
DIM = 256
GP = 125              # window size (25 chunks)
NSC = SEQ // GP       # 32
NG = 8                # groups of 4 windows
GW = 505              # group buffer width (500 + 5 shift overlap)
F32 = mybir.dt.float32
F32R = mybir.dt.float32r
BF16 = mybir.dt.bfloat16
g6 = float(np.float64(GAMMA) ** 6)
COPY = mybir.ActivationFunctionType.Copy

# const blob column layout (f32)
C_WIT = 0             # [0:125)   intra mask, shifted coords
C_WCT = 125           # [125:250) 0/1 cross mask, shifted coords
C_Z = 250             # [250:762) zeros (row 0: zero matmul operands)
C_END = 762


def make_const_blob():
    j = np.arange(GP)
    jj, rr = j[:, None], j[None, :]
    witn = np.where((jj // B == rr // B) & (rr % B >= jj % B),
                    np.float64(GAMMA) ** (jj % B - rr % B), 0.0)
    wctn = (jj // B <= rr // B).astype(np.float64)
    blob = np.zeros((128, C_END), np.float32)
    blob[0:GP, C_WIT:C_WIT + GP] = witn.astype(np.float32)
    blob[0:GP, C_WCT:C_WCT + GP] = wctn.astype(np.float32)
    return blob


def build_kernel(nc: bass.Bass):
    xqT = nc.dram_tensor("xqT", [FEAT, SEQ], BF16, kind="ExternalInput").ap()
    xkT = nc.dram_tensor("xkT", [FEAT, SEQ], BF16, kind="ExternalInput").ap()
    xvT = nc.dram_tensor("xvT", [FEAT, SEQ], BF16, kind="ExternalInput").ap()
    wqkv = nc.dram_tensor("wqkv", [FEAT, 3 * DIM], BF16, kind="ExternalInput").ap()
    out = nc.dram_tensor("out", [SEQ, DIM], F32, kind="ExternalOutput").ap()

    blob_np = make_const_blob()
    ident_np = np.eye(128, dtype=ml_dtypes.bfloat16)
    mm = nc.tensor.matmul

    with tile.TileContext(nc) as tc:
        with (
            tc.tile_pool(name="consts", bufs=1) as cpool,
            tc.tile_pool(name="xin", bufs=3) as xpool,
            tc.tile_pool(name="qkt", bufs=2) as qpool,
            tc.tile_pool(name="work", bufs=2) as spool,
            tc.tile_pool(name="psT", bufs=1, space="PSUM") as psT,
            tc.tile_pool(name="psV", bufs=1, space="PSUM") as psV,
            tc.tile_pool(name="psX", bufs=2, space="PSUM") as psX,
            tc.tile_pool(name="psW", bufs=2, space="PSUM") as psW,
            tc.tile_pool(name="psU", bufs=1, space="PSUM") as psU,
        ):
            # weights DMA first (first projection gates on it; wq alone
            # unblocks the first projection), then group-0 x loads, then the
            # const blob / identity, then group 1.
            w_sb = cpool.tile([128, 2, 3 * DIM], BF16, name="w_sb")
            nc.sync.dma_start(out=w_sb[:, :, 0:256],
                              in_=wqkv[:, 0:256].rearrange("(h p) d -> p h d", p=128))
            blob_sb = cpool.tile([128, C_END], F32, name="blob_sb")
            ident_sb = cpool.tile([128, 128], BF16, name="ident_sb")
            wit_sb = blob_sb[0:GP, C_WIT:C_WIT + GP]
            ww_sb = blob_sb[0:GP, 0:2 * GP].rearrange("p (b c) -> p b c", b=2)

            u_ps = psU.tile([128, 512], F32, name="u_state")

            xg = {}
            qts = {}
            kts = {}

            def load_group_x(g):
                tiles = []
                for nm, src in (("xq", xqT), ("xk", xkT), ("xv", xvT)):
                    t = xpool.tile([128, 2, GW], BF16, name=f"{nm}_{g}", tag=nm)
                    if g == 0:
                        nc.vector.memset(t[:, :, 0:5], 0.0)
                        nc.sync.dma_start(
                            out=t[:, :, 5:GW],
                            in_=src[:, 0:500].rearrange("(h p) a -> p h a", p=128))
                    else:
                        nc.sync.dma_start(
                            out=t,
                            in_=src[:, g * 500 - 5:g * 500 + 500]
                                .rearrange("(h p) a -> p h a", p=128))
                    tiles.append(t)
                xg[g] = tiles

            def proj_qkt(g, which):
                x = xg[g][0 if which == "qt" else 1]
                dlos = (0, 128) if which == "qt" else (256, 384)
                ps = psT.tile([128, 1024], F32, name=f"ps_{which}_{g}", tag="qkt")
                for off, dlo in ((0, dlos[0]), (512, dlos[1])):
                    for h in (0, 1):
                        mm(ps[:, off:off + GW], w_sb[:, h, dlo:dlo + 128],
                           x[:, h, :], start=(h == 0), stop=(h == 1))
                t = qpool.tile([128, 2, GW], BF16, name=f"{which}_{g}", tag=which)
                ps_v = ps.rearrange("p (b c) -> p b c", b=2)[:, :, 0:GW]
                nc.scalar.activation(t, ps_v, COPY)
                (qts if which == "qt" else kts)[g] = t

            def vproj(s):
                g, ls = divmod(s, 4)
                xv = xg[g][2]
                vs = psV.tile([GP, 256], F32, name=f"v_{s}", tag="v")
                for h in (0, 1):
                    mm(vs, xv[:, h, ls * GP:ls * GP + GP], w_sb[:, h, 512:768],
                       start=(h == 0), stop=(h == 1))
                v_sb = spool.tile([GP, 256], BF16, name=f"vsb_{s}", tag="vsb")
                nc.vector.tensor_copy(v_sb, vs)
                return v_sb

            def ptmm(s):
                # P~^T in cols 0:130 (f32) of a [125, 512] tile whose bytes
                # 1536:2048 also hold the K-transpose output (bf16 bitcast) —
                # one bank, two bufs, so the mask chain isn't serialized.
                g, ls = divmod(s, 4)
                qt, kt = qts[g], kts[g]
                px = psX.tile([GP, 512], F32, name=f"px_{s}", tag="px")
                for h in (0, 1):
                    mm(px[:, 0:130], kt[:, h, ls * GP:ls * GP + GP],
                       qt[:, h, ls * GP:ls * GP + 130],
                       start=(h == 0), stop=(h == 1))
                return px

            def ktrans(s, px):
                g, ls = divmod(s, 4)
                kt = kts[g]
                kv = px[:, 384:512].bitcast(BF16)   # [125, 256] bf16 region
                mm(kv[:, 0:128], kt[:, 0, ls * GP:ls * GP + GP], ident_sb,
                   is_transpose=True, skip_group_check=True)
                mm(kv[:, 128:256], kt[:, 1, ls * GP:ls * GP + GP], ident_sb,
                   is_transpose=True, skip_group_check=True)
                k_sb = spool.tile([GP, 256], BF16, name=f"ksb_{s}", tag="ksb",
                                  bufs=3)
                nc.vector.tensor_copy(k_sb, kv)
                return k_sb

            def masks(s, px):
                # one DVE mul: in0 = pt at col offsets {0, 5} (2-block AP),
                # in1 = [wit | wct] blob block, out = both products; the
                # SBUF-only add runs on the otherwise-idle Pool engine.
                c2 = spool.tile([GP, 2, GP], BF16, name=f"c2_{s}", tag="c2")
                pt_blocks = bass.AP(px.tensor, px.offset,
                                    [[512, GP], [5, 2], [1, GP]])
                nc.vector.tensor_mul(c2, pt_blocks, ww_sb)
                comb = spool.tile([GP, GP], BF16, name=f"comb_{s}", tag="comb",
                                  bufs=3)
                nc.gpsimd.tensor_add(comb, c2[:, 0, :], c2[:, 1, :])
                return comb

            def tail():
                # chunk 799 (positions 3995:4000), intra-only
                xv7 = xg[7][2]
                v5ps = psV.tile([5, 256], F32, name="v5", tag="v")
                for h in (0, 1):
                    mm(v5ps, xv7[:, h, 500:GW], w_sb[:, h, 512:768],
                       start=(h == 0), stop=(h == 1))
                v5_sb = spool.tile([5, 256], BF16, name="v5sb", tag="v5sb")
                nc.vector.tensor_copy(v5_sb, v5ps)
                px5 = psX.tile([GP, 512], F32, name="px5", tag="px")
                for h in (0, 1):
                    mm(px5[0:5, 0:5], kts[7][:, h, 500:GW],
                       qts[7][:, h, 500:GW], start=(h == 0), stop=(h == 1))
                c5 = spool.tile([5, 5], BF16, name="c5", tag="c5")
                nc.vector.tensor_mul(c5, px5[0:5, 0:5], blob_sb[0:5, C_WIT:C_WIT + 5])
                wtf = psW.tile([GP, 256], F32, name="wtf", tag="wt")
                mm(wtf[0:5, :], c5, v5_sb, start=True, stop=True)
                wallf = spool.tile([5, 256], F32, name="wallf", tag="wallf")
                nc.scalar.activation(wallf, wtf[0:5, :], COPY)
                nc.sync.dma_start(out=out[SEQ - 5:SEQ], in_=wallf)

            # --- prologue ---
            load_group_x(0)
            nc.sync.dma_start(out=w_sb[:, :, 256:768],
                              in_=wqkv[:, 256:768].rearrange("(h p) d -> p h d", p=128))
            nc.sync.dma_start(out=blob_sb, in_=nc.inline_tensor(blob_np, "cblob").ap())
            nc.sync.dma_start(out=ident_sb,
                              in_=nc.inline_tensor(ident_np, "cident").ap())
            load_group_x(1)

            proj_qkt(0, "qt")
            proj_qkt(0, "kt")
            # zero-matmul sets the U bank's data + has_written bits so state
            # matmuls accumulate; scr/scr2 absorb the blob/ident DMA waits.
            mm(u_ps, blob_sb[0:1, C_Z:C_Z + 128].bitcast(F32R),
               blob_sb[0:1, C_Z:C_Z + 512].bitcast(F32R),
               start=True, stop=True, skip_group_check=True)
            scr = spool.tile([1, 1], F32, name="scr", tag="scr")
            nc.vector.tensor_copy(scr, blob_sb[0:1, 0:1])
            scr2 = spool.tile([1, 1], BF16, name="scr2", tag="scr2")
            nc.scalar.activation(scr2, ident_sb[0:1, 0:1], COPY)
            vs = {0: vproj(0)}
            ks = {}
            combs = {}
            for t in (0, 1):
                pxt = ptmm(t)
                ks[t] = ktrans(t, pxt)
                combs[t] = masks(t, pxt)
            ut_prev = None
            wt_pair = {}

            for s in range(NSC):
                g, ls = divmod(s, 4)
                # P~^T(s+2) + its DVE mask mul go FIRST (PE and DVE program
                # order) so the mask chain for comb(s+2) starts two
                # iterations early and never gates a window matmul.
                px = ptmm(s + 2) if s + 2 < NSC else None
                if px is not None:
                    combs[s + 2] = masks(s + 2, px)
                if s + 1 < NSC:
                    vs[s + 1] = vproj(s + 1)

                wt = psW.tile([GP, 256], F32, name=f"wt_{s}", tag="wt")
                mm(wt, combs[s], vs[s], start=True, stop=(s == 0))
                if s > 0:
                    qt = qts[g]
                    mm(wt, qt[:, 0, ls * GP + 5:ls * GP + 130], ut_prev[:, 0:256],
                       start=False, stop=False)
                    mm(wt, qt[:, 1, ls * GP + 5:ls * GP + 130],
                       ut_prev[:, 256:512], start=False, stop=True)

                mm(u_ps[:, 0:256], ks[s][:, 0:128], vs[s],
                   start=False, stop=True, skip_group_check=True)
                mm(u_ps[:, 256:512], ks[s][:, 128:256], vs[s],
                   start=False, stop=True, skip_group_check=True)
                if s + 1 < NSC:
                    ut_prev = spool.tile([128, 512], BF16, name=f"ut_{s}", tag="ut")
                    nc.scalar.activation(ut_prev, u_ps, COPY)

                if ls == 0 and g + 1 < NG:
                    proj_qkt(g + 1, "qt")
                if ls == 1 and g + 1 < NG:
                    proj_qkt(g + 1, "kt")
                    if g + 2 < NG:
                        load_group_x(g + 2)

                if px is not None:
                    ks[s + 2] = ktrans(s + 2, px)
                if s == 26:
                    tail()

                if s % 2 == 0:
                    wall2 = spool.tile([GP, 2, 256], F32, name=f"wall_{s}",
                                       tag="wall", bufs=2)
                    wt_pair[0] = wall2
                    nc.vector.tensor_copy(wall2[:, 0, :], wt)
                else:
                    wall2 = wt_pair[0]
                    nc.scalar.activation(wall2[:, 1, :], wt, COPY)
                    if s == 1:
                        nc.sync.dma_start(out=out[0:GP - 5],
                                          in_=wall2[5:GP, 0, :])
                        nc.sync.dma_start(out=out[GP - 5:2 * GP - 5],
                                          in_=wall2[:, 1, :])
                    else:
                        nc.sync.dma_start(
                            out=out[(s - 1) * GP - 5:(s + 1) * GP - 5]
                                .rearrange("(b p) d -> p b d", b=2),
                            in_=wall2)
                vs.pop(s, None)
                ks.pop(s, None)
                combs.pop(s, None)

    return nc


def _col_scales():
    j = np.arange(SEQ) // B          # global chunk index
    sq = (np.float64(g6) ** j).astype(np.float32)
    sk = (np.float64(g6) ** (-j)).astype(np.float32)
    return sq, sk


def prep_core_inputs(xq2d, xk2d, xv2d, wqkv):
    sq, sk = _col_scales()
    return {
        "xqT": (xq2d.T * sq[None, :]).astype(ml_dtypes.bfloat16),
        "xkT": (xk2d.T * sk[None, :]).astype(ml_dtypes.bfloat16),
        "xvT": np.ascontiguousarray(xv2d.T).astype(ml_dtypes.bfloat16),
        "wqkv": wqkv.astype(ml_dtypes.bfloat16),
    }


def make_in_maps(inputs):
    """inputs: dict from setup_inputs (full batch). Returns per-core in_maps."""
    xq, xk, xv = inputs["xq"], inputs["xk"], inputs["xv"]
    wqkv = np.ascontiguousarray(np.concatenate(
        [np.asarray(inputs["Wq"], dtype=np.float32),
         np.asarray(inputs["Wk"], dtype=np.float32),
         np.asarray(inputs["Wv"], dtype=np.float32)], axis=1))
    in_maps = []
    for b in range(8):
        in_maps.append(prep_core_inputs(
            np.asarray(xq[b], dtype=np.float32),
            np.asarray(xk[b], dtype=np.float32),
            np.asarray(xv[b], dtype=np.float32), wqkv))
    return in_maps


_NC_CACHE = {}


def _get_nc():
    if "nc" not in _NC_CACHE:
        from concourse import bacc
        nc = bacc.Bacc("TRN2", target_bir_lowering=False, debug=False)
        build_kernel(nc)
        nc.compile()
        _NC_CACHE["nc"] = nc
    return _NC_CACHE["nc"]


def run(inputs, trace=False, **kwargs):
    """Run on 8 NeuronCores; returns (output [8,4000,256], BassKernelResults)."""
    from concourse.bass_utils import run_bass_kernel_spmd

    nc = _get_nc()
    in_maps = make_in_maps(inputs)
    res = run_bass_kernel_spmd(nc, in_maps, core_ids=list(range(8)),
                               trace=trace, **kwargs)
    out = np.stack([r["out"] for r in res.results], axis=0)
    return out, res


def kernel(**inputs) -> np.ndarray:
    out, _ = run(inputs)
    return out


# revision 12
# speedup vs baseline: 1.0630x; 1.0200x over previous
"""Bass/Tile kernel for chunkwise retention (nn_ChunkwiseRetention).

Shifted-window scheme (v2), per core = one batch element, seq 4000, B=5:

Windows of 125 positions shifted by -5: window s covers output positions
[s*125-5, s*125+120), and the V/K contraction range is the SAME shifted
span, so the seam (intra of the chunk straddling the superchunk boundary)
folds into the single combined masked matmul — no separate seam matmul.
The carry boundary moves one chunk earlier: carry_s = Q[s*125:+125] @
U_shift(s-1) with U_shift accumulating K^T V over shifted windows.

Host pre-scales xqT columns by g6^j and xkT by g6^-j (j = chunk index),
folding all cross-chunk decay into the projections (cross mask is 0/1).
All inputs and SBUF operands are bf16 (PE: 1 cycle/row at any moving
width, halved DMA); PSUM accumulation stays f32.

Per iteration s: V proj (s+1, shifted window), window matmuls for s
(comb + carry into one PSUM group), P~^T (s+1) at N=130 (q cols shifted
-5..+125), state update, group-ahead Q^T/K^T projections (N=505, fused
single PSUM->SBUF copy), K pos-major via PE transposes into a bf16
bitcast region of the same PSUM tile as P~^T (shared bank, bufs=2 so
the mask chain has a full iteration of slack), one DVE mask mul via a
2-block strided AP + SBUF-only add on the Pool engine, paired output
DMAs. Tail (chunk 799, intra-only) is issued early at s==27.

PSUM banks (8): qkt 2 + v 1 + (pt|ktr) 2 + wt 2 + u 1.
"""
import numpy as np
import ml_dtypes

import concourse.bass as bass
import concourse.mybir as mybir
import concourse.tile as tile

GAMMA = 0.9865
B = 5
SEQ = 4000
FEAT = 256
DIM = 256
GP = 125              # window size (25 chunks)
NSC = SEQ // GP       # 32
NG = 8                # groups of 4 windows
GW = 505              # group buffer width (500 + 5 shift overlap)
F32 = mybir.dt.float32
F32R = mybir.dt.float32r
BF16 = mybir.dt.bfloat16
g6 = float(np.float64(GAMMA) ** 6)
COPY = mybir.ActivationFunctionType.Copy

# const blob column layout (f32)
C_WIT = 0             # [0:125)   intra mask, shifted coords
C_WCT = 125           # [125:250) 0/1 cross mask, shifted coords
C_Z = 250             # [250:762) zeros (row 0: zero matmul operands)
C_END = 762


def make_const_blob():
    j = np.arange(GP)
    jj, rr = j[:, None], j[None, :]
    witn = np.where((jj // B == rr // B) & (rr % B >= jj % B),
                    np.float64(GAMMA) ** (jj % B - rr % B), 0.0)
    wctn = (jj // B <= rr // B).astype(np.float64)
    blob = np.zeros((128, C_END), np.float32)
    blob[0:GP, C_WIT:C_WIT + GP] = witn.astype(np.float32)
    blob[0:GP, C_WCT:C_WCT + GP] = wctn.astype(np.float32)
    return blob


def build_kernel(nc: bass.Bass):
    xqT = nc.dram_tensor("xqT", [FEAT, SEQ], BF16, kind="ExternalInput").ap()
    xkT = nc.dram_tensor("xkT", [FEAT, SEQ], BF16, kind="ExternalInput").ap()
    xvT = nc.dram_tensor("xvT", [FEAT, SEQ], BF16, kind="ExternalInput").ap()
    wqkv = nc.dram_tensor("wqkv", [FEAT, 3 * DIM], BF16, kind="ExternalInput").ap()
    out = nc.dram_tensor("out", [SEQ, DIM], F32, kind="ExternalOutput").ap()

    blob_np = make_const_blob()
    ident_np = np.eye(128, dtype=ml_dtypes.bfloat16)
    mm = nc.tensor.matmul

    with tile.TileContext(nc) as tc:
        with (
            tc.tile_pool(name="consts", bufs=1) as cpool,
            tc.tile_pool(name="xin", bufs=3) as xpool,
            tc.tile_pool(name="qkt", bufs=2) as qpool,
            tc.tile_pool(name="work", bufs=2) as spool,
            tc.tile_pool(name="psT", bufs=1, space="PSUM") as psT,
            tc.tile_pool(name="psV", bufs=1, space="PSUM") as psV,
            tc.tile_pool(name="psX", bufs=2, space="PSUM") as psX,
            tc.tile_pool(name="psW", bufs=2, space="PSUM") as psW,
            tc.tile_pool(name="psU", bufs=1, space="PSUM") as psU,
        ):
            # weights DMA first (first projection gates on it; wq alone
            # unblocks the first projection), then group-0 x loads, then the
            # const blob / identity, then group 1.
            w_sb = cpool.tile([128, 2, 3 * DIM], BF16, name="w_sb")
            nc.sync.dma_start(out=w_sb[:, :, 0:256],
                              in_=wqkv[:, 0:256].rearrange("(h p) d -> p h d", p=128))
            blob_sb = cpool.tile([128, C_END], F32, name="blob_sb")
            ident_sb = cpool.tile([128, 128], BF16, name="ident_sb")
            wit_sb = blob_sb[0:GP, C_WIT:C_WIT + GP]
            ww_sb = blob_sb[0:GP, 0:2 * GP].rearrange("p (b c) -> p b c", b=2)

            u_ps = psU.tile([128, 512], F32, name="u_state")

            xg = {}
            qts = {}
            kts = {}

            def load_group_x(g):
                tiles = []
                for nm, src in (("xq", xqT), ("xk", xkT), ("xv", xvT)):
                    t = xpool.tile([128, 2, GW], BF16, name=f"{nm}_{g}", tag=nm)
                    if g == 0:
                        nc.vector.memset(t[:, :, 0:5], 0.0)
                        nc.sync.dma_start(
                            out=t[:, :, 5:GW],
                            in_=src[:, 0:500].rearrange("(h p) a -> p h a", p=128))
                    else:
                        nc.sync.dma_start(
                            out=t,
                            in_=src[:, g * 500 - 5:g * 500 + 500]
                                .rearrange("(h p) a -> p h a", p=128))
                    tiles.append(t)
                xg[g] = tiles

            def proj_qkt(g, which):
                x = xg[g][0 if which == "qt" else 1]
                dlos = (0, 128) if which == "qt" else (256, 384)
                ps = psT.tile([128, 1024], F32, name=f"ps_{which}_{g}", tag="qkt")
                for off, dlo in ((0, dlos[0]), (512, dlos[1])):
                    for h in (0, 1):
                        mm(ps[:, off:off + GW], w_sb[:, h, dlo:dlo + 128],
                           x[:, h, :], start=(h == 0), stop=(h == 1))
                t = qpool.tile([128, 2, GW], BF16, name=f"{which}_{g}", tag=which)
                ps_v = ps.rearrange("p (b c) -> p b c", b=2)[:, :, 0:GW]
                nc.scalar.activation(t, ps_v, COPY)
                (qts if which == "qt" else kts)[g] = t

            def vproj(s):
                g, ls = divmod(s, 4)
                xv = xg[g][2]
                vs = psV.tile([GP, 256], F32, name=f"v_{s}", tag="v")
                for h in (0, 1):
                    mm(vs, xv[:, h, ls * GP:ls * GP + GP], w_sb[:, h, 512:768],
                       start=(h == 0), stop=(h == 1))
                v_sb = spool.tile([GP, 256], BF16, name=f"vsb_{s}", tag="vsb")
                nc.vector.tensor_copy(v_sb, vs)
                return v_sb

            def ptmm(s):
                # P~^T in cols 0:130 (f32) of a [125, 512] tile whose bytes
                # 1536:2048 also hold the K-transpose output (bf16 bitcast) —
                # one bank, two bufs, so the mask chain isn't serialized.
                g, ls = divmod(s, 4)
                qt, kt = qts[g], kts[g]
                px = psX.tile([GP, 512], F32, name=f"px_{s}", tag="px")
                for h in (0, 1):
                    mm(px[:, 0:130], kt[:, h, ls * GP:ls * GP + GP],
                       qt[:, h, ls * GP:ls * GP + 130],
                       start=(h == 0), stop=(h == 1))
                return px

            def ktrans(s, px):
                g, ls = divmod(s, 4)
                kt = kts[g]
                kv = px[:, 384:512].bitcast(BF16)   # [125, 256] bf16 region
                mm(kv[:, 0:128], kt[:, 0, ls * GP:ls * GP + GP], ident_sb,
                   is_transpose=True, skip_group_check=True)
                mm(kv[:, 128:256], kt[:, 1, ls * GP:ls * GP + GP], ident_sb,
                   is_transpose=True, skip_group_check=True)
                k_sb = spool.tile([GP, 256], BF16, name=f"ksb_{s}", tag="ksb",
                                  bufs=3)
                nc.vector.tensor_copy(k_sb, kv)
                return k_sb

            def masks(s, px):
                # one DVE mul: in0 = pt at col offsets {0, 5} (2-block AP),
                # in1 = [wit | wct] blob block, out = both products; the
                # SBUF-only add runs on the otherwise-idle Pool engine.
                c2 = spool.tile([GP, 2, GP], BF16, name=f"c2_{s}", tag="c2")
                pt_blocks = bass.AP(px.tensor, px.offset,
                                    [[512, GP], [5, 2], [1, GP]])
                nc.vector.tensor_mul(c2, pt_blocks, ww_sb)
                comb = spool.tile([GP, GP], BF16, name=f"comb_{s}", tag="comb",
                                  bufs=3)
                nc.gpsimd.tensor_add(comb, c2[:, 0, :], c2[:, 1, :])
                return comb

            def tail():
                # chunk 799 (positions 3995:4000), intra-only
                xv7 = xg[7][2]
                v5ps = psV.tile([5, 256], F32, name="v5", tag="v")
                for h in (0, 1):
                    mm(v5ps, xv7[:, h, 500:GW], w_sb[:, h, 512:768],
                       start=(h == 0), stop=(h == 1))
                v5_sb = spool.tile([5, 256], BF16, name="v5sb", tag="v5sb")
                nc.vector.tensor_copy(v5_sb, v5ps)
                px5 = psX.tile([GP, 512], F32, name="px5", tag="px")
                for h in (0, 1):
                    mm(px5[0:5, 0:5], kts[7][:, h, 500:GW],
                       qts[7][:, h, 500:GW], start=(h == 0), stop=(h == 1))
                c5 = spool.tile([5, 5], BF16, name="c5", tag="c5")
                nc.vector.tensor_mul(c5, px5[0:5, 0:5], blob_sb[0:5, C_WIT:C_WIT + 5])
                wtf = psW.tile([GP, 256], F32, name="wtf", tag="wt")
                mm(wtf[0:5, :], c5, v5_sb, start=True, stop=True)
                wallf = spool.tile([5, 256], F32, name="wallf", tag="wallf")
                nc.scalar.activation(wallf, wtf[0:5, :], COPY)
                nc.sync.dma_start(out=out[SEQ - 5:SEQ], in_=wallf)

            # --- prologue ---
            # PE p-state warmup: a tiny matmul at t~100 starts the 3us ramp
            # clock so the first real projections run at full speed.
            warm = spool.tile([1, 2], BF16, name="warm", tag="warm")
            nc.vector.memset(warm, 0.0)
            mm(u_ps[0:1, 0:2], warm[0:1, 0:1], warm,
               start=True, stop=True, skip_group_check=True)
            load_group_x(0)
            nc.sync.dma_start(out=w_sb[:, :, 256:768],
                              in_=wqkv[:, 256:768].rearrange("(h p) d -> p h d", p=128))
            nc.sync.dma_start(out=blob_sb, in_=nc.inline_tensor(blob_np, "cblob").ap())
            nc.sync.dma_start(out=ident_sb,
                              in_=nc.inline_tensor(ident_np, "cident").ap())
            load_group_x(1)

            proj_qkt(0, "qt")
            proj_qkt(0, "kt")
            proj_qkt(1, "qt")
            # zero-matmul sets the U bank's data + has_written bits so state
            # matmuls accumulate; scr/scr2 absorb the blob/ident DMA waits.
            mm(u_ps, blob_sb[0:1, C_Z:C_Z + 128].bitcast(F32R),
               blob_sb[0:1, C_Z:C_Z + 512].bitcast(F32R),
               start=True, stop=True, skip_group_check=True)
            scr = spool.tile([1, 1], F32, name="scr", tag="scr")
            nc.vector.tensor_copy(scr, blob_sb[0:1, 0:1])
            scr2 = spool.tile([1, 1], BF16, name="scr2", tag="scr2")
            nc.scalar.activation(scr2, ident_sb[0:1, 0:1], COPY)
            vs = {0: vproj(0)}
            ks = {}
            combs = {}
            for t in (0, 1):
                pxt = ptmm(t)
                ks[t] = ktrans(t, pxt)
                combs[t] = masks(t, pxt)
            ut_prev = None
            wt_pair = {}

            for s in range(NSC):
                g, ls = divmod(s, 4)
                # V(s+1) first (v-copy leads the DVE queue; psV frees for
                # the next iteration), then P~^T(s+2) + its mask mul so the
                # mask chain for comb(s+2) starts two iterations early.
                if s + 1 < NSC:
                    vs[s + 1] = vproj(s + 1)
                px = ptmm(s + 2) if s + 2 < NSC else None
                if px is not None:
                    combs[s + 2] = masks(s + 2, px)

                wt = psW.tile([GP, 256], F32, name=f"wt_{s}", tag="wt")
                mm(wt, combs[s], vs[s], start=True, stop=(s == 0))
                if s > 0:
                    qt = qts[g]
                    mm(wt, qt[:, 0, ls * GP + 5:ls * GP + 130], ut_prev[:, 0:256],
                       start=False, stop=False)
                    mm(wt, qt[:, 1, ls * GP + 5:ls * GP + 130],
                       ut_prev[:, 256:512], start=False, stop=True)

                mm(u_ps[:, 0:256], ks[s][:, 0:128], vs[s],
                   start=False, stop=True, skip_group_check=True)
                mm(u_ps[:, 256:512], ks[s][:, 128:256], vs[s],
                   start=False, stop=True, skip_group_check=True)
                if s + 1 < NSC:
                    ut_prev = spool.tile([128, 512], BF16, name=f"ut_{s}", tag="ut")
                    nc.scalar.activation(ut_prev, u_ps, COPY)

                if ls == 3 and g + 2 < NG:
                    proj_qkt(g + 2, "qt")
                if ls == 0 and g + 1 < NG:
                    proj_qkt(g + 1, "kt")
                    if g + 2 < NG:
                        load_group_x(g + 2)

                if px is not None:
                    ks[s + 2] = ktrans(s + 2, px)
                if s == 26:
                    tail()

                if s % 2 == 0:
                    wall2 = spool.tile([GP, 2, 256], F32, name=f"wall_{s}",
                                       tag="wall", bufs=2)
                    wt_pair[0] = wall2
                    nc.vector.tensor_copy(wall2[:, 0, :], wt)
                else:
                    wall2 = wt_pair[0]
                    nc.scalar.activation(wall2[:, 1, :], wt, COPY)
                    if s == 1:
                        nc.sync.dma_start(out=out[0:GP - 5],
                                          in_=wall2[5:GP, 0, :])
                        nc.sync.dma_start(out=out[GP - 5:2 * GP - 5],
                                          in_=wall2[:, 1, :])
                    else:
                        nc.sync.dma_start(
                            out=out[(s - 1) * GP - 5:(s + 1) * GP - 5]
                                .rearrange("(b p) d -> p b d", b=2),
                            in_=wall2)
                vs.pop(s, None)
                ks.pop(s, None)
                combs.pop(s, None)

    return nc


def _col_scales():
    j = np.arange(SEQ) // B          # global chunk index
    sq = (np.float64(g6) ** j).astype(np.float32)
    sk = (np.float64(g6) ** (-j)).astype(np.float32)
    return sq, sk


def prep_core_inputs(xq2d, xk2d, xv2d, wqkv):
    sq, sk = _col_scales()
    return {
        "xqT": (xq2d.T * sq[None, :]).astype(ml_dtypes.bfloat16),
        "xkT": (xk2d.T * sk[None, :]).astype(ml_dtypes.bfloat16),
        "xvT": np.ascontiguousarray(xv2d.T).astype(ml_dtypes.bfloat16),
        "wqkv": wqkv.astype(ml_dtypes.bfloat16),
    }


def make_in_maps(inputs):
    """inputs: dict from setup_inputs (full batch). Returns per-core in_maps."""
    xq, xk, xv = inputs["xq"], inputs["xk"], inputs["xv"]
    wqkv = np.ascontiguousarray(np.concatenate(
        [np.asarray(inputs["Wq"], dtype=np.float32),
         np.asarray(inputs["Wk"], dtype=np.float32),
         np.asarray(inputs["Wv"], dtype=np.float32)], axis=1))
    in_maps = []
    for b in range(8):
        in_maps.append(prep_core_inputs(
            np.asarray(xq[b], dtype=np.float32),
            np.asarray(xk[b], dtype=np.float32),
            np.asarray(xv[b], dtype=np.float32), wqkv))
    return in_maps


_NC_CACHE = {}


def _get_nc():
    if "nc" not in _NC_CACHE:
        from concourse import bacc
        nc = bacc.Bacc("TRN2", target_bir_lowering=False, debug=False)
        build_kernel(nc)
        nc.compile()
        _NC_CACHE["nc"] = nc
    return _NC_CACHE["nc"]


def run(inputs, trace=False, **kwargs):
    """Run on 8 NeuronCores; returns (output [8,4000,256], BassKernelResults)."""
    from concourse.bass_utils import run_bass_kernel_spmd

    nc = _get_nc()
    in_maps = make_in_maps(inputs)
    res = run_bass_kernel_spmd(nc, in_maps, core_ids=list(range(8)),
                               trace=trace, **kwargs)
    out = np.stack([r["out"] for r in res.results], axis=0)
    return out, res


def kernel(**inputs) -> np.ndarray:
    out, _ = run(inputs)
    return out


# revision 13
# speedup vs baseline: 1.0727x; 1.0091x over previous
"""Bass/Tile kernel for chunkwise retention (nn_ChunkwiseRetention).

Shifted-window scheme (v2), per core = one batch element, seq 4000, B=5:

Windows of 125 positions shifted by -5: window s covers output positions
[s*125-5, s*125+120), and the V/K contraction range is the SAME shifted
span, so the seam (intra of the chunk straddling the superchunk boundary)
folds into the single combined masked matmul — no separate seam matmul.
The carry boundary moves one chunk earlier: carry_s = Q[s*125:+125] @
U_shift(s-1) with U_shift accumulating K^T V over shifted windows.

Host pre-scales xqT columns by g6^j and xkT by g6^-j (j = chunk index),
folding all cross-chunk decay into the projections (cross mask is 0/1).
All inputs and SBUF operands are bf16 (PE: 1 cycle/row at any moving
width, halved DMA); PSUM accumulation stays f32.

Per iteration s: V proj (s+1, shifted window), window matmuls for s
(comb + carry into one PSUM group), P~^T (s+1) at N=130 (q cols shifted
-5..+125), state update, group-ahead Q^T/K^T projections (N=505, fused
single PSUM->SBUF copy), K pos-major via PE transposes into a bf16
bitcast region of the same PSUM tile as P~^T (shared bank, bufs=2 so
the mask chain has a full iteration of slack), one DVE mask mul via a
2-block strided AP + SBUF-only add on the Pool engine, paired output
DMAs. Tail (chunk 799, intra-only) is issued early at s==27.

PSUM banks (8): qkt 2 + v 1 + (pt|ktr) 2 + wt 2 + u 1.
"""
import numpy as np
import ml_dtypes

import concourse.bass as bass
import concourse.mybir as mybir
import concourse.tile as tile

GAMMA = 0.9865
B = 5
SEQ = 4000
FEAT = 256
DIM = 256
GP = 125              # window size (25 chunks)
NSC = SEQ // GP       # 32
NG = 8                # groups of 4 windows
GW = 505              # group buffer width (500 + 5 shift overlap)
F32 = mybir.dt.float32
F32R = mybir.dt.float32r
BF16 = mybir.dt.bfloat16
g6 = float(np.float64(GAMMA) ** 6)
COPY = mybir.ActivationFunctionType.Copy

# const blob column layout (f32)
C_WIT = 0             # [0:125)   intra mask, shifted coords
C_WCT = 125           # [125:250) 0/1 cross mask, shifted coords
C_Z = 250             # [250:762) zeros (row 0: zero matmul operands)
C_END = 762


def make_const_blob():
    j = np.arange(GP)
    jj, rr = j[:, None], j[None, :]
    witn = np.where((jj // B == rr // B) & (rr % B >= jj % B),
                    np.float64(GAMMA) ** (jj % B - rr % B), 0.0)
    wctn = (jj // B <= rr // B).astype(np.float64)
    blob = np.zeros((128, C_END), np.float32)
    blob[0:GP, C_WIT:C_WIT + GP] = witn.astype(np.float32)
    blob[0:GP, C_WCT:C_WCT + GP] = wctn.astype(np.float32)
    return blob


def build_kernel(nc: bass.Bass):
    xqT = nc.dram_tensor("xqT", [FEAT, SEQ], BF16, kind="ExternalInput").ap()
    xkT = nc.dram_tensor("xkT", [FEAT, SEQ], BF16, kind="ExternalInput").ap()
    xvT = nc.dram_tensor("xvT", [FEAT, SEQ], BF16, kind="ExternalInput").ap()
    wqkv = nc.dram_tensor("wqkv", [FEAT, 3 * DIM], BF16, kind="ExternalInput").ap()
    out = nc.dram_tensor("out", [SEQ, DIM], F32, kind="ExternalOutput").ap()

    blob_np = make_const_blob()
    ident_np = np.eye(128, dtype=ml_dtypes.bfloat16)
    mm = nc.tensor.matmul

    with tile.TileContext(nc) as tc:
        with (
            tc.tile_pool(name="consts", bufs=1) as cpool,
            tc.tile_pool(name="xin", bufs=3) as xpool,
            tc.tile_pool(name="qkt", bufs=2) as qpool,
            tc.tile_pool(name="work", bufs=2) as spool,
            tc.tile_pool(name="psT", bufs=1, space="PSUM") as psT,
            tc.tile_pool(name="psV", bufs=1, space="PSUM") as psV,
            tc.tile_pool(name="psX", bufs=2, space="PSUM") as psX,
            tc.tile_pool(name="psW", bufs=2, space="PSUM") as psW,
            tc.tile_pool(name="psU", bufs=1, space="PSUM") as psU,
        ):
            # weights DMA first (first projection gates on it; wq alone
            # unblocks the first projection), then group-0 x loads, then the
            # const blob / identity, then group 1.
            w_sb = cpool.tile([128, 2, 3 * DIM], BF16, name="w_sb")
            nc.sync.dma_start(out=w_sb[:, :, 0:256],
                              in_=wqkv[:, 0:256].rearrange("(h p) d -> p h d", p=128))
            blob_sb = cpool.tile([128, C_END], F32, name="blob_sb")
            ident_sb = cpool.tile([128, 128], BF16, name="ident_sb")
            wit_sb = blob_sb[0:GP, C_WIT:C_WIT + GP]
            ww_sb = blob_sb[0:GP, 0:2 * GP].rearrange("p (b c) -> p b c", b=2)

            u_ps = psU.tile([128, 512], F32, name="u_state")

            xg = {}
            qts = {}
            kts = {}

            def load_group_x(g):
                tiles = []
                for nm, src in (("xq", xqT), ("xk", xkT), ("xv", xvT)):
                    t = xpool.tile([128, 2, GW], BF16, name=f"{nm}_{g}", tag=nm)
                    if g == 0:
                        nc.vector.memset(t[:, :, 0:5], 0.0)
                        nc.sync.dma_start(
                            out=t[:, :, 5:GW],
                            in_=src[:, 0:500].rearrange("(h p) a -> p h a", p=128))
                    else:
                        nc.sync.dma_start(
                            out=t,
                            in_=src[:, g * 500 - 5:g * 500 + 500]
                                .rearrange("(h p) a -> p h a", p=128))
                    tiles.append(t)
                xg[g] = tiles

            pend = {}

            def proj_mms(g, which):
                x = xg[g][0 if which == "qt" else 1]
                dlos = (0, 128) if which == "qt" else (256, 384)
                ps = psT.tile([128, 1024], F32, name=f"ps_{which}_{g}", tag="qkt")
                for off, dlo in ((0, dlos[0]), (512, dlos[1])):
                    for h in (0, 1):
                        mm(ps[:, off:off + GW], w_sb[:, h, dlo:dlo + 128],
                           x[:, h, :], start=(h == 0), stop=(h == 1))
                pend[which, g] = ps

            def proj_copy(g, which):
                ps = pend.pop((which, g))
                t = qpool.tile([128, 2, GW], BF16, name=f"{which}_{g}", tag=which)
                ps_v = ps.rearrange("p (b c) -> p b c", b=2)[:, :, 0:GW]
                nc.scalar.activation(t, ps_v, COPY)
                (qts if which == "qt" else kts)[g] = t

            def proj_qkt(g, which):
                proj_mms(g, which)
                proj_copy(g, which)

            def vproj(s):
                g, ls = divmod(s, 4)
                xv = xg[g][2]
                vs = psV.tile([GP, 256], F32, name=f"v_{s}", tag="v")
                for h in (0, 1):
                    mm(vs, xv[:, h, ls * GP:ls * GP + GP], w_sb[:, h, 512:768],
                       start=(h == 0), stop=(h == 1))
                v_sb = spool.tile([GP, 256], BF16, name=f"vsb_{s}", tag="vsb")
                nc.vector.tensor_copy(v_sb, vs)
                return v_sb

            def ptmm(s):
                # P~^T in cols 0:130 (f32) of a [125, 512] tile whose bytes
                # 1536:2048 also hold the K-transpose output (bf16 bitcast) —
                # one bank, two bufs, so the mask chain isn't serialized.
                g, ls = divmod(s, 4)
                qt, kt = qts[g], kts[g]
                px = psX.tile([GP, 512], F32, name=f"px_{s}", tag="px")
                for h in (0, 1):
                    mm(px[:, 0:130], kt[:, h, ls * GP:ls * GP + GP],
                       qt[:, h, ls * GP:ls * GP + 130],
                       start=(h == 0), stop=(h == 1))
                return px

            def ktrans(s, px, on_act):
                g, ls = divmod(s, 4)
                kt = kts[g]
                kv = px[:, 384:512].bitcast(BF16)   # [125, 256] bf16 region
                mm(kv[:, 0:128], kt[:, 0, ls * GP:ls * GP + GP], ident_sb,
                   is_transpose=True, skip_group_check=True)
                mm(kv[:, 128:256], kt[:, 1, ls * GP:ls * GP + GP], ident_sb,
                   is_transpose=True, skip_group_check=True)
                k_sb = spool.tile([GP, 256], BF16, name=f"ksb_{s}", tag="ksb",
                                  bufs=3)
                if on_act:
                    nc.scalar.activation(k_sb, kv, COPY)
                else:
                    nc.vector.tensor_copy(k_sb, kv)
                return k_sb

            def masks(s, px):
                # one DVE mul: in0 = pt at col offsets {0, 5} (2-block AP),
                # in1 = [wit | wct] blob block, out = both products; the
                # SBUF-only add runs on the otherwise-idle Pool engine.
                c2 = spool.tile([GP, 2, GP], BF16, name=f"c2_{s}", tag="c2")
                pt_blocks = bass.AP(px.tensor, px.offset,
                                    [[512, GP], [5, 2], [1, GP]])
                nc.vector.tensor_mul(c2, pt_blocks, ww_sb)
                comb = spool.tile([GP, GP], BF16, name=f"comb_{s}", tag="comb",
                                  bufs=3)
                nc.gpsimd.tensor_add(comb, c2[:, 0, :], c2[:, 1, :])
                return comb

            def tail():
                # chunk 799 (positions 3995:4000), intra-only
                xv7 = xg[7][2]
                v5ps = psV.tile([5, 256], F32, name="v5", tag="v")
                for h in (0, 1):
                    mm(v5ps, xv7[:, h, 500:GW], w_sb[:, h, 512:768],
                       start=(h == 0), stop=(h == 1))
                v5_sb = spool.tile([5, 256], BF16, name="v5sb", tag="v5sb")
                nc.vector.tensor_copy(v5_sb, v5ps)
                px5 = psX.tile([GP, 512], F32, name="px5", tag="px")
                for h in (0, 1):
                    mm(px5[0:5, 0:5], kts[7][:, h, 500:GW],
                       qts[7][:, h, 500:GW], start=(h == 0), stop=(h == 1))
                c5 = spool.tile([5, 5], BF16, name="c5", tag="c5")
                nc.vector.tensor_mul(c5, px5[0:5, 0:5], blob_sb[0:5, C_WIT:C_WIT + 5])
                wtf = psW.tile([GP, 256], F32, name="wtf", tag="wt")
                mm(wtf[0:5, :], c5, v5_sb, start=True, stop=True)
                wallf = spool.tile([5, 256], F32, name="wallf", tag="wallf")
                nc.scalar.activation(wallf, wtf[0:5, :], COPY)
                nc.sync.dma_start(out=out[SEQ - 5:SEQ], in_=wallf)

            # --- prologue ---
            # PE p-state warmup: a tiny matmul at t~100 starts the 3us ramp
            # clock so the first real projections run at full speed.
            warm = spool.tile([1, 2], BF16, name="warm", tag="warm")
            nc.vector.memset(warm, 0.0)
            mm(u_ps[0:1, 0:2], warm[0:1, 0:1], warm,
               start=True, stop=True, skip_group_check=True)
            load_group_x(0)
            nc.sync.dma_start(out=w_sb[:, :, 256:768],
                              in_=wqkv[:, 256:768].rearrange("(h p) d -> p h d", p=128))
            nc.sync.dma_start(out=blob_sb, in_=nc.inline_tensor(blob_np, "cblob").ap())
            nc.sync.dma_start(out=ident_sb,
                              in_=nc.inline_tensor(ident_np, "cident").ap())
            load_group_x(1)

            proj_qkt(0, "qt")
            proj_qkt(0, "kt")
            proj_qkt(1, "qt")
            # zero-matmul sets the U bank's data + has_written bits so state
            # matmuls accumulate; scr/scr2 absorb the blob/ident DMA waits.
            mm(u_ps, blob_sb[0:1, C_Z:C_Z + 128].bitcast(F32R),
               blob_sb[0:1, C_Z:C_Z + 512].bitcast(F32R),
               start=True, stop=True, skip_group_check=True)
            scr = spool.tile([1, 1], F32, name="scr", tag="scr")
            nc.vector.tensor_copy(scr, blob_sb[0:1, 0:1])
            scr2 = spool.tile([1, 1], BF16, name="scr2", tag="scr2")
            nc.scalar.activation(scr2, ident_sb[0:1, 0:1], COPY)
            vs = {0: vproj(0)}
            ks = {}
            combs = {}
            for t in (0, 1):
                pxt = ptmm(t)
                ks[t] = ktrans(t, pxt, on_act=(t % 2 == 1))
                combs[t] = masks(t, pxt)
            ut_prev = None
            wt_pair = {}

            for s in range(NSC):
                g, ls = divmod(s, 4)
                # V(s+1) first (v-copy leads the DVE queue; psV frees for
                # the next iteration), then P~^T(s+2) + its mask mul so the
                # mask chain for comb(s+2) starts two iterations early.
                if s + 1 < NSC:
                    vs[s + 1] = vproj(s + 1)
                px = ptmm(s + 2) if s + 2 < NSC else None
                if px is not None:
                    combs[s + 2] = masks(s + 2, px)

                wt = psW.tile([GP, 256], F32, name=f"wt_{s}", tag="wt")
                mm(wt, combs[s], vs[s], start=True, stop=(s == 0))
                if s > 0:
                    qt = qts[g]
                    mm(wt, qt[:, 0, ls * GP + 5:ls * GP + 130], ut_prev[:, 0:256],
                       start=False, stop=False)
                    mm(wt, qt[:, 1, ls * GP + 5:ls * GP + 130],
                       ut_prev[:, 256:512], start=False, stop=True)

                mm(u_ps[:, 0:256], ks[s][:, 0:128], vs[s],
                   start=False, stop=True, skip_group_check=True)
                mm(u_ps[:, 256:512], ks[s][:, 128:256], vs[s],
                   start=False, stop=True, skip_group_check=True)
                if s + 1 < NSC:
                    ut_prev = spool.tile([128, 512], BF16, name=f"ut_{s}", tag="ut")
                    nc.scalar.activation(ut_prev, u_ps, COPY)

                if ls == 0:
                    if g + 1 >= 2 and g + 1 < NG:
                        proj_copy(g + 1, "qt")
                    if g + 1 < NG:
                        proj_mms(g + 1, "kt")
                    if g + 2 < NG:
                        load_group_x(g + 2)
                if ls == 1 and g + 1 < NG:
                    proj_copy(g + 1, "kt")
                if ls == 3 and g + 2 < NG:
                    proj_mms(g + 2, "qt")

                if px is not None:
                    ks[s + 2] = ktrans(s + 2, px, on_act=(ls >= 2))
                if s == 26:
                    tail()

                if s % 2 == 0:
                    wall2 = spool.tile([GP, 2, 256], F32, name=f"wall_{s}",
                                       tag="wall", bufs=2)
                    wt_pair[0] = wall2
                    nc.vector.tensor_copy(wall2[:, 0, :], wt)
                else:
                    wall2 = wt_pair[0]
                    nc.vector.tensor_copy(wall2[:, 1, :], wt)
                    if s == 1:
                        nc.sync.dma_start(out=out[0:GP - 5],
                                          in_=wall2[5:GP, 0, :])
                        nc.sync.dma_start(out=out[GP - 5:2 * GP - 5],
                                          in_=wall2[:, 1, :])
                    else:
                        nc.sync.dma_start(
                            out=out[(s - 1) * GP - 5:(s + 1) * GP - 5]
                                .rearrange("(b p) d -> p b d", b=2),
                            in_=wall2)
                vs.pop(s, None)
                ks.pop(s, None)
                combs.pop(s, None)

    return nc


def _col_scales():
    j = np.arange(SEQ) // B          # global chunk index
    sq = (np.float64(g6) ** j).astype(np.float32)
    sk = (np.float64(g6) ** (-j)).astype(np.float32)
    return sq, sk


def prep_core_inputs(xq2d, xk2d, xv2d, wqkv):
    sq, sk = _col_scales()
    return {
        "xqT": (xq2d.T * sq[None, :]).astype(ml_dtypes.bfloat16),
        "xkT": (xk2d.T * sk[None, :]).astype(ml_dtypes.bfloat16),
        "xvT": np.ascontiguousarray(xv2d.T).astype(ml_dtypes.bfloat16),
        "wqkv": wqkv.astype(ml_dtypes.bfloat16),
    }


def make_in_maps(inputs):
    """inputs: dict from setup_inputs (full batch). Returns per-core in_maps."""
    xq, xk, xv = inputs["xq"], inputs["xk"], inputs["xv"]
    wqkv = np.ascontiguousarray(np.concatenate(
        [np.asarray(inputs["Wq"], dtype=np.float32),
         np.asarray(inputs["Wk"], dtype=np.float32),
         np.asarray(inputs["Wv"], dtype=np.float32)], axis=1))
    in_maps = []
    for b in range(8):
        in_maps.append(prep_core_inputs(
            np.asarray(xq[b], dtype=np.float32),
            np.asarray(xk[b], dtype=np.float32),
            np.asarray(xv[b], dtype=np.float32), wqkv))
    return in_maps


_NC_CACHE = {}


def _get_nc():
    if "nc" not in _NC_CACHE:
        from concourse import bacc
        nc = bacc.Bacc("TRN2", target_bir_lowering=False, debug=False)
        build_kernel(nc)
        nc.compile()
        _NC_CACHE["nc"] = nc
    return _NC_CACHE["nc"]


def run(inputs, trace=False, **kwargs):
    """Run on 8 NeuronCores; returns (output [8,4000,256], BassKernelResults)."""
    from concourse.bass_utils import run_bass_kernel_spmd

    nc = _get_nc()
    in_maps = make_in_maps(inputs)
    res = run_bass_kernel_spmd(nc, in_maps, core_ids=list(range(8)),
                               trace=trace, **kwargs)
    out = np.stack([r["out"] for r in res.results], axis=0)
    return out, res


def kernel(**inputs) -> np.ndarray:
    out, _ = run(inputs)
    return out


# revision 14
# speedup vs baseline: 1.1039x; 1.0291x over previous
"""Bass/Tile kernel for chunkwise retention (nn_ChunkwiseRetention).

Shifted-window scheme (v2), per core = one batch element, seq 4000, B=5:

Windows of 125 positions shifted by -5: window s covers output positions
[s*125-5, s*125+120), and the V/K contraction range is the SAME shifted
span, so the seam (intra of the chunk straddling the superchunk boundary)
folds into the single combined masked matmul — no separate seam matmul.
The carry boundary moves one chunk earlier: carry_s = Q[s*125:+125] @
U_shift(s-1) with U_shift accumulating K^T V over shifted windows.

Host pre-scales xqT columns by g6^j and xkT by g6^-j (j = chunk index),
folding all cross-chunk decay into the projections (cross mask is 0/1).
All inputs and SBUF operands are bf16 (PE: 1 cycle/row at any moving
width, halved DMA); PSUM accumulation stays f32.

Per iteration s: V proj (s+1, shifted window), window matmuls for s
(comb + carry into one PSUM group), P~^T (s+1) at N=130 (q cols shifted
-5..+125), state update, group-ahead Q^T/K^T projections (N=505, fused
single PSUM->SBUF copy), K pos-major via PE transposes into a bf16
bitcast region of the same PSUM tile as P~^T (shared bank, bufs=2 so
the mask chain has a full iteration of slack), one DVE mask mul via a
2-block strided AP + SBUF-only add on the Pool engine, paired output
DMAs. Tail (chunk 799, intra-only) is issued early at s==27.

PSUM banks (8): qkt 2 + v 1 + (pt|ktr) 2 + wt 2 + u 1.
"""
import numpy as np
import ml_dtypes

import concourse.bass as bass
import concourse.mybir as mybir
import concourse.tile as tile

GAMMA = 0.9865
B = 5
SEQ = 4000
FEAT = 256
DIM = 256
GP = 125              # window size (25 chunks)
NSC = SEQ // GP       # 32
NG = 8                # groups of 4 windows
GW = 505              # group buffer width (500 + 5 shift overlap)
F32 = mybir.dt.float32
F32R = mybir.dt.float32r
BF16 = mybir.dt.bfloat16
g6 = float(np.float64(GAMMA) ** 6)
COPY = mybir.ActivationFunctionType.Copy

# const blob column layout (f32)
C_WIT = 0             # [0:125)   intra mask, shifted coords
C_WCT = 125           # [125:250) 0/1 cross mask, shifted coords
C_Z = 250             # [250:762) zeros (row 0: zero matmul operands)
C_END = 762


def make_const_blob():
    j = np.arange(GP)
    jj, rr = j[:, None], j[None, :]
    witn = np.where((jj // B == rr // B) & (rr % B >= jj % B),
                    np.float64(GAMMA) ** (jj % B - rr % B), 0.0)
    wctn = (jj // B <= rr // B).astype(np.float64)
    blob = np.zeros((128, C_END), np.float32)
    blob[0:GP, C_WIT:C_WIT + GP] = witn.astype(np.float32)
    blob[0:GP, C_WCT:C_WCT + GP] = wctn.astype(np.float32)
    return blob


def build_kernel(nc: bass.Bass):
    xqT = nc.dram_tensor("xqT", [FEAT, SEQ], BF16, kind="ExternalInput").ap()
    xkT = nc.dram_tensor("xkT", [FEAT, SEQ], BF16, kind="ExternalInput").ap()
    xvT = nc.dram_tensor("xvT", [FEAT, SEQ], BF16, kind="ExternalInput").ap()
    wqkv = nc.dram_tensor("wqkv", [FEAT, 3 * DIM], BF16, kind="ExternalInput").ap()
    out = nc.dram_tensor("out", [SEQ, DIM], F32, kind="ExternalOutput").ap()

    blob_np = make_const_blob()
    ident_np = np.eye(128, dtype=ml_dtypes.bfloat16)
    mm = nc.tensor.matmul

    with tile.TileContext(nc) as tc:
        with (
            tc.tile_pool(name="consts", bufs=1) as cpool,
            tc.tile_pool(name="xin", bufs=3) as xpool,
            tc.tile_pool(name="qkt", bufs=2) as qpool,
            tc.tile_pool(name="work", bufs=2) as spool,
            tc.tile_pool(name="psT", bufs=1, space="PSUM") as psT,
            tc.tile_pool(name="psV", bufs=2, space="PSUM") as psV,
            tc.tile_pool(name="psX", bufs=2, space="PSUM") as psX,
            tc.tile_pool(name="psW", bufs=1, space="PSUM") as psW,
            tc.tile_pool(name="psU", bufs=1, space="PSUM") as psU,
        ):
            # weights DMA first (first projection gates on it; wq alone
            # unblocks the first projection), then group-0 x loads, then the
            # const blob / identity, then group 1.
            w_sb = cpool.tile([128, 2, 3 * DIM], BF16, name="w_sb")
            nc.sync.dma_start(out=w_sb[:, :, 0:256],
                              in_=wqkv[:, 0:256].rearrange("(h p) d -> p h d", p=128))
            blob_sb = cpool.tile([128, C_END], F32, name="blob_sb")
            ident_sb = cpool.tile([128, 128], BF16, name="ident_sb")
            wit_sb = blob_sb[0:GP, C_WIT:C_WIT + GP]
            ww_sb = blob_sb[0:GP, 0:2 * GP].rearrange("p (b c) -> p b c", b=2)

            u_ps = psU.tile([128, 512], F32, name="u_state")

            xg = {}
            qts = {}
            kts = {}

            def load_group_x(g):
                tiles = []
                for nm, src in (("xq", xqT), ("xk", xkT), ("xv", xvT)):
                    t = xpool.tile([128, 2, GW], BF16, name=f"{nm}_{g}", tag=nm)
                    if g == 0:
                        nc.vector.memset(t[:, :, 0:5], 0.0)
                        nc.sync.dma_start(
                            out=t[:, :, 5:GW],
                            in_=src[:, 0:500].rearrange("(h p) a -> p h a", p=128))
                    else:
                        nc.sync.dma_start(
                            out=t,
                            in_=src[:, g * 500 - 5:g * 500 + 500]
                                .rearrange("(h p) a -> p h a", p=128))
                    tiles.append(t)
                xg[g] = tiles

            pend = {}

            def proj_mms(g, which):
                x = xg[g][0 if which == "qt" else 1]
                dlos = (0, 128) if which == "qt" else (256, 384)
                ps = psT.tile([128, 1024], F32, name=f"ps_{which}_{g}", tag="qkt")
                for off, dlo in ((0, dlos[0]), (512, dlos[1])):
                    for h in (0, 1):
                        mm(ps[:, off:off + GW], w_sb[:, h, dlo:dlo + 128],
                           x[:, h, :], start=(h == 0), stop=(h == 1))
                pend[which, g] = ps

            def proj_copy(g, which):
                ps = pend.pop((which, g))
                t = qpool.tile([128, 2, GW], BF16, name=f"{which}_{g}", tag=which)
                ps_v = ps.rearrange("p (b c) -> p b c", b=2)[:, :, 0:GW]
                nc.scalar.activation(t, ps_v, COPY)
                (qts if which == "qt" else kts)[g] = t

            def proj_qkt(g, which):
                proj_mms(g, which)
                proj_copy(g, which)

            def vproj(s):
                g, ls = divmod(s, 4)
                xv = xg[g][2]
                vs = psV.tile([GP, 256], F32, name=f"v_{s}", tag="v")
                for h in (0, 1):
                    mm(vs, xv[:, h, ls * GP:ls * GP + GP], w_sb[:, h, 512:768],
                       start=(h == 0), stop=(h == 1))
                v_sb = spool.tile([GP, 256], BF16, name=f"vsb_{s}", tag="vsb")
                nc.vector.tensor_copy(v_sb, vs)
                return v_sb

            def ptmm(s):
                # P~^T in cols 0:130 (f32) of a [125, 512] tile whose bytes
                # 1536:2048 also hold the K-transpose output (bf16 bitcast) —
                # one bank, two bufs, so the mask chain isn't serialized.
                g, ls = divmod(s, 4)
                qt, kt = qts[g], kts[g]
                px = psX.tile([GP, 512], F32, name=f"px_{s}", tag="px")
                for h in (0, 1):
                    mm(px[:, 0:130], kt[:, h, ls * GP:ls * GP + GP],
                       qt[:, h, ls * GP:ls * GP + 130],
                       start=(h == 0), stop=(h == 1))
                return px

            def ktrans(s, px, on_act):
                g, ls = divmod(s, 4)
                kt = kts[g]
                kv = px[:, 384:512].bitcast(BF16)   # [125, 256] bf16 region
                mm(kv[:, 0:128], kt[:, 0, ls * GP:ls * GP + GP], ident_sb,
                   is_transpose=True, skip_group_check=True)
                mm(kv[:, 128:256], kt[:, 1, ls * GP:ls * GP + GP], ident_sb,
                   is_transpose=True, skip_group_check=True)
                k_sb = spool.tile([GP, 256], BF16, name=f"ksb_{s}", tag="ksb",
                                  bufs=3)
                if on_act:
                    nc.scalar.activation(k_sb, kv, COPY)
                else:
                    nc.vector.tensor_copy(k_sb, kv)
                return k_sb

            def masks(s, px):
                # one DVE mul: in0 = pt at col offsets {0, 5} (2-block AP),
                # in1 = [wit | wct] blob block, out = both products; the
                # SBUF-only add runs on the otherwise-idle Pool engine.
                c2 = spool.tile([GP, 2, GP], BF16, name=f"c2_{s}", tag="c2")
                pt_blocks = bass.AP(px.tensor, px.offset,
                                    [[512, GP], [5, 2], [1, GP]])
                nc.vector.tensor_mul(c2, pt_blocks, ww_sb)
                comb = spool.tile([GP, GP], BF16, name=f"comb_{s}", tag="comb",
                                  bufs=3)
                nc.gpsimd.tensor_add(comb, c2[:, 0, :], c2[:, 1, :])
                return comb

            def tail():
                # chunk 799 (positions 3995:4000), intra-only
                xv7 = xg[7][2]
                v5ps = psV.tile([5, 256], F32, name="v5", tag="v")
                for h in (0, 1):
                    mm(v5ps, xv7[:, h, 500:GW], w_sb[:, h, 512:768],
                       start=(h == 0), stop=(h == 1))
                v5_sb = spool.tile([5, 256], BF16, name="v5sb", tag="v5sb")
                nc.vector.tensor_copy(v5_sb, v5ps)
                px5 = psX.tile([GP, 512], F32, name="px5", tag="px")
                for h in (0, 1):
                    mm(px5[0:5, 0:5], kts[7][:, h, 500:GW],
                       qts[7][:, h, 500:GW], start=(h == 0), stop=(h == 1))
                c5 = spool.tile([5, 5], BF16, name="c5", tag="c5")
                nc.vector.tensor_mul(c5, px5[0:5, 0:5], blob_sb[0:5, C_WIT:C_WIT + 5])
                wtf = psV.tile([5, 256], F32, name="wtf", tag="v")
                mm(wtf, c5, v5_sb, start=True, stop=True)
                wallf = spool.tile([5, 256], F32, name="wallf", tag="wallf")
                nc.scalar.activation(wallf, wtf, COPY)
                nc.sync.dma_start(out=out[SEQ - 5:SEQ], in_=wallf)

            # --- prologue ---
            # PE p-state warmup: a tiny matmul at t~100 starts the 3us ramp
            # clock so the first real projections run at full speed.
            warm = spool.tile([1, 2], BF16, name="warm", tag="warm")
            nc.vector.memset(warm, 0.0)
            mm(u_ps[0:1, 0:2], warm[0:1, 0:1], warm,
               start=True, stop=True, skip_group_check=True)
            load_group_x(0)
            nc.sync.dma_start(out=w_sb[:, :, 256:768],
                              in_=wqkv[:, 256:768].rearrange("(h p) d -> p h d", p=128))
            nc.sync.dma_start(out=blob_sb, in_=nc.inline_tensor(blob_np, "cblob").ap())
            nc.sync.dma_start(out=ident_sb,
                              in_=nc.inline_tensor(ident_np, "cident").ap())
            load_group_x(1)

            proj_qkt(0, "qt")
            proj_qkt(0, "kt")
            proj_qkt(1, "qt")
            # zero-matmul sets the U bank's data + has_written bits so state
            # matmuls accumulate; scr/scr2 absorb the blob/ident DMA waits.
            mm(u_ps, blob_sb[0:1, C_Z:C_Z + 128].bitcast(F32R),
               blob_sb[0:1, C_Z:C_Z + 512].bitcast(F32R),
               start=True, stop=True, skip_group_check=True)
            scr = spool.tile([1, 1], F32, name="scr", tag="scr")
            nc.vector.tensor_copy(scr, blob_sb[0:1, 0:1])
            scr2 = spool.tile([1, 1], BF16, name="scr2", tag="scr2")
            nc.scalar.activation(scr2, ident_sb[0:1, 0:1], COPY)
            vs = {0: vproj(0)}
            ks = {}
            combs = {}
            for t in (0, 1):
                pxt = ptmm(t)
                ks[t] = ktrans(t, pxt, on_act=(t % 2 == 1))
                combs[t] = masks(t, pxt)
            ut_prev = None
            wt_pair = {}
            wtp_ps = psW.tile([GP, 512], F32, name="wt_pair_ps")

            for s in range(NSC):
                g, ls = divmod(s, 4)
                # V(s+1) first (v-copy leads the DVE queue; psV frees for
                # the next iteration), then P~^T(s+2) + its mask mul so the
                # mask chain for comb(s+2) starts two iterations early.
                if s + 1 < NSC:
                    vs[s + 1] = vproj(s + 1)
                px = ptmm(s + 2) if s + 2 < NSC else None
                if px is not None:
                    combs[s + 2] = masks(s + 2, px)

                wt = wtp_ps[:, (s % 2) * 256:(s % 2) * 256 + 256]
                mm(wt, combs[s], vs[s], start=True, stop=(s == 0),
                   skip_group_check=True)

                mm(u_ps[:, 0:256], ks[s][:, 0:128], vs[s],
                   start=False, stop=True, skip_group_check=True)
                mm(u_ps[:, 256:512], ks[s][:, 128:256], vs[s],
                   start=False, stop=True, skip_group_check=True)
                ut_carry = ut_prev
                if s + 1 < NSC:
                    ut_prev = spool.tile([128, 512], BF16, name=f"ut_{s}", tag="ut")
                    nc.scalar.activation(ut_prev, u_ps, COPY)

                if ls == 0:
                    if g + 1 >= 2 and g + 1 < NG:
                        proj_copy(g + 1, "qt")
                    if g + 1 < NG:
                        proj_mms(g + 1, "kt")
                    if g + 2 < NG:
                        load_group_x(g + 2)
                if ls == 1 and g + 1 < NG:
                    proj_copy(g + 1, "kt")
                if ls == 3 and g + 2 < NG:
                    proj_mms(g + 2, "qt")

                if px is not None:
                    ks[s + 2] = ktrans(s + 2, px, on_act=(ls >= 2))
                if s == 26:
                    tail()

                if s > 0:
                    qt = qts[g]
                    mm(wt, qt[:, 0, ls * GP + 5:ls * GP + 130], ut_carry[:, 0:256],
                       start=False, stop=False, skip_group_check=True)
                    mm(wt, qt[:, 1, ls * GP + 5:ls * GP + 130],
                       ut_carry[:, 256:512], start=False, stop=True,
                       skip_group_check=True)

                if s % 2 == 0:
                    wall2 = spool.tile([GP, 2, 256], F32, name=f"wall_{s}",
                                       tag="wall", bufs=2)
                    wt_pair[0] = wall2
                    nc.vector.tensor_copy(wall2[:, 0, :], wt)
                else:
                    wall2 = wt_pair[0]
                    nc.vector.tensor_copy(wall2[:, 1, :], wt)
                    if s == 1:
                        nc.sync.dma_start(out=out[0:GP - 5],
                                          in_=wall2[5:GP, 0, :])
                        nc.sync.dma_start(out=out[GP - 5:2 * GP - 5],
                                          in_=wall2[:, 1, :])
                    else:
                        nc.sync.dma_start(
                            out=out[(s - 1) * GP - 5:(s + 1) * GP - 5]
                                .rearrange("(b p) d -> p b d", b=2),
                            in_=wall2)
                vs.pop(s, None)
                ks.pop(s, None)
                combs.pop(s, None)

    return nc


def _col_scales():
    j = np.arange(SEQ) // B          # global chunk index
    sq = (np.float64(g6) ** j).astype(np.float32)
    sk = (np.float64(g6) ** (-j)).astype(np.float32)
    return sq, sk


def prep_core_inputs(xq2d, xk2d, xv2d, wqkv):
    sq, sk = _col_scales()
    return {
        "xqT": (xq2d.T * sq[None, :]).astype(ml_dtypes.bfloat16),
        "xkT": (xk2d.T * sk[None, :]).astype(ml_dtypes.bfloat16),
        "xvT": np.ascontiguousarray(xv2d.T).astype(ml_dtypes.bfloat16),
        "wqkv": wqkv.astype(ml_dtypes.bfloat16),
    }


def make_in_maps(inputs):
    """inputs: dict from setup_inputs (full batch). Returns per-core in_maps."""
    xq, xk, xv = inputs["xq"], inputs["xk"], inputs["xv"]
    wqkv = np.ascontiguousarray(np.concatenate(
        [np.asarray(inputs["Wq"], dtype=np.float32),
         np.asarray(inputs["Wk"], dtype=np.float32),
         np.asarray(inputs["Wv"], dtype=np.float32)], axis=1))
    in_maps = []
    for b in range(8):
        in_maps.append(prep_core_inputs(
            np.asarray(xq[b], dtype=np.float32),
            np.asarray(xk[b], dtype=np.float32),
            np.asarray(xv[b], dtype=np.float32), wqkv))
    return in_maps


_NC_CACHE = {}


def _get_nc():
    if "nc" not in _NC_CACHE:
        from concourse import bacc
        nc = bacc.Bacc("TRN2", target_bir_lowering=False, debug=False)
        build_kernel(nc)
        nc.compile()
        _NC_CACHE["nc"] = nc
    return _NC_CACHE["nc"]


def run(inputs, trace=False, **kwargs):
    """Run on 8 NeuronCores; returns (output [8,4000,256], BassKernelResults)."""
    from concourse.bass_utils import run_bass_kernel_spmd

    nc = _get_nc()
    in_maps = make_in_maps(inputs)
    res = run_bass_kernel_spmd(nc, in_maps, core_ids=list(range(8)),
                               trace=trace, **kwargs)
    out = np.stack([r["out"] for r in res.results], axis=0)
    return out, res


def kernel(**inputs) -> np.ndarray:
    out, _ = run(inputs)
    return out
